# revision 7
# baseline (speedup 1.0000x reference)
"""Two-layer SAGEConv (mean aggregation) GNN on 8 trn2 NeuronCores.

Strategy (dst-sharded graph parallel, v2):
  - dst nodes are packed into 320 bins of 128 slots (40 bins per core) by a
    degree-balanced snake-LPT, so every bin holds <= 2048 edges. Each bin is
    one psum "range"; its edges fit 16 blocks of 128 edge-slots (2.5% pad
    instead of the 28% of the fixed 5000-row layout).
  - Each core gets its OWN permuted feature table [40960, 128] bf16 split in
    two 20480-row halves (int16 gather indices). Sources are assigned to the
    A/B halves per core by a greedy first-appearance rule that balances each
    bin's edge split, so every bin statically uses 8 A-blocks + 8 B-blocks.
  - Messages x[src] are fetched with dma_gather (16 calls of 5120 rows).
    The 0/1 routing ("one-hot") matrices are generated ON-CHIP from 1-byte
    slot targets via broadcast is_equal (DVE + Pool engines) instead of
    streaming ~13MB of one-hot data from HBM.
  - Aggregation per range: psum[f, slot] += msg_blk[e, f].T @ onehot[e, slot]
    over its 16 blocks; then mean scale (1/deg along free dim), linear
    hT = act(W_l.T @ meanT + W_r.T @ xT + b). Layer 1 PE-transposes hT back
    to row-major to rebuild the h table (host assembles between launches).
"""
import numpy as np
import ml_dtypes
from contextlib import ExitStack

import concourse.bass as bass
import concourse.mybir as mybir
import concourse.tile as tile
from concourse import bacc
from concourse.library_config import mlp
from concourse import bass_utils

BF16 = mybir.dt.bfloat16
F32 = mybir.dt.float32
I16 = mybir.dt.int16
NP_BF16 = ml_dtypes.bfloat16

N = 40000
D = 128
CORES = 8
RANGES = 40            # bins (dst ranges of 128 slots) per core
NPAD = RANGES * 128    # 5120 dst positions per core
NBINS = CORES * RANGES
HALF_ROWS = 20480      # rows per table half (int16-indexable)
RPG = 5                # ranges per gather/onehot group
GROUPS = RANGES // RPG

_prog_cache = {}


def build_program(layer, KA=8):
    """One SPMD program for one SAGEConv layer. KA = blocks per (range, half)."""
    BPR = 2 * KA                    # blocks per range
    NBLK_G = RPG * KA               # blocks per (group, half)
    GN = NBLK_G * 128               # gathered idxs per call
    GC = GN // 16                   # wrapped idx cols per call
    IDX_COLS = GROUPS * GC
    TBLK = RANGES * KA              # blocks per half per layer

    nc = bacc.Bacc("TRN2", target_bir_lowering=False, debug=False)
    table = nc.dram_tensor("table", [2 * HALF_ROWS, D], BF16, kind="ExternalInput")
    idxA_d = nc.dram_tensor("idxA", [128, IDX_COLS], I16, kind="ExternalInput")
    idxB_d = nc.dram_tensor("idxB", [128, IDX_COLS], I16, kind="ExternalInput")
    tgtA_d = nc.dram_tensor("tgtA", [128, TBLK], BF16, kind="ExternalInput")
    tgtB_d = nc.dram_tensor("tgtB", [128, TBLK], BF16, kind="ExternalInput")
    iota_d = nc.dram_tensor("iota", [128, 128], BF16, kind="ExternalInput")
    xT_d = nc.dram_tensor("xT", [128, NPAD], BF16, kind="ExternalInput")
    recip_d = nc.dram_tensor("recipb", [128, NPAD], BF16, kind="ExternalInput")
    Wl_d = nc.dram_tensor("Wl", [128, 128], BF16, kind="ExternalInput")
    Wr_d = nc.dram_tensor("Wr", [128, 128], BF16, kind="ExternalInput")
    b_d = nc.dram_tensor("bvec", [128, 1], F32, kind="ExternalInput")
    if layer == 1:
        ident_d = nc.dram_tensor("ident", [128, 128], BF16, kind="ExternalInput")
        hout = nc.dram_tensor("hout", [NPAD, D], BF16, kind="ExternalOutput")
    else:
        tout = nc.dram_tensor("tout", [128, NPAD], F32, kind="ExternalOutput")

    with tile.TileContext(nc) as tc, ExitStack() as ctx:
        const = ctx.enter_context(tc.tile_pool(name="const", bufs=1))
        pmA = ctx.enter_context(tc.tile_pool(name="msgA", bufs=2))
        pmB = ctx.enter_context(tc.tile_pool(name="msgB", bufs=2))
        pohA = ctx.enter_context(tc.tile_pool(name="ohA", bufs=2))
        pohB = ctx.enter_context(tc.tile_pool(name="ohB", bufs=2))
        psagg = ctx.enter_context(tc.tile_pool(name="psagg", bufs=3, space="PSUM"))
        pslin = ctx.enter_context(tc.tile_pool(name="pslin", bufs=2, space="PSUM"))
        pmean = ctx.enter_context(tc.tile_pool(name="mean", bufs=3))
        if layer == 1:
            pstr = ctx.enter_context(tc.tile_pool(name="pstr", bufs=2, space="PSUM"))
            phT = ctx.enter_context(tc.tile_pool(name="hT", bufs=3))

        nc.gpsimd.load_library(mlp)

        idxA = const.tile([128, IDX_COLS], I16)
        nc.sync.dma_start(idxA[:], idxA_d[:])
        idxB = const.tile([128, IDX_COLS], I16)
        nc.sync.dma_start(idxB[:], idxB_d[:])
        tgtA = const.tile([128, TBLK], BF16)
        nc.sync.dma_start(tgtA[:], tgtA_d[:])
        tgtB = const.tile([128, TBLK], BF16)
        nc.sync.dma_start(tgtB[:], tgtB_d[:])
        iota = const.tile([128, 128], BF16)
        nc.sync.dma_start(iota[:], iota_d[:])
        # iotaE[p, f, b] = f — materialized so the is_equal gen below has
        # stride-1 last dims on every operand (qualifies for DVE 2x mode)
        iotaE = const.tile([128, 128, NBLK_G], BF16)
        nc.vector.tensor_copy(iotaE[:],
                              iota[:, :, None].to_broadcast([128, 128, NBLK_G]))
        xT = const.tile([128, NPAD], BF16)
        nc.sync.dma_start(xT[:], xT_d[:])
        recip = const.tile([128, NPAD], BF16)
        nc.sync.dma_start(recip[:], recip_d[:])
        Wl = const.tile([128, 128], BF16)
        nc.sync.dma_start(Wl[:], Wl_d[:])
        Wr = const.tile([128, 128], BF16)
        nc.sync.dma_start(Wr[:], Wr_d[:])
        bv = const.tile([128, 1], F32)
        nc.sync.dma_start(bv[:], b_d[:])
        if layer == 1:
            ident = const.tile([128, 128], BF16)
            nc.sync.dma_start(ident[:], ident_d[:])
            hstage = const.tile([128, RANGES, 128], BF16)
        else:
            ostage = const.tile([128, NPAD], F32)

        tabA = table[0:HALF_ROWS, :]
        tabB = table[HALF_ROWS:2 * HALF_ROWS, :]

        for g in range(GROUPS):
            msgA = pmA.tile([128, NBLK_G, 128], BF16)
            nc.gpsimd.dma_gather(msgA[:], tabA, idxA[:, g * GC:(g + 1) * GC],
                                 GN, GN, D, single_packet=False)
            msgB = pmB.tile([128, NBLK_G, 128], BF16)
            nc.gpsimd.dma_gather(msgB[:], tabB, idxB[:, g * GC:(g + 1) * GC],
                                 GN, GN, D, single_packet=False)
            # routing matrices, slot-major: R[e, slot, blk] = (tgt[e, blk] == slot)
            # (TensorTensor is only HW-legal on DVE; this layout keeps every
            # operand's last dim stride-1 so the DVE 2x perf mode applies)
            ohA = pohA.tile([128, 128, NBLK_G], BF16)
            nc.vector.tensor_tensor(
                out=ohA[:],
                in0=tgtA[:, None, g * NBLK_G:(g + 1) * NBLK_G]
                .to_broadcast([128, 128, NBLK_G]),
                in1=iotaE[:],
                op=mybir.AluOpType.is_equal)
            ohB = pohB.tile([128, 128, NBLK_G], BF16)
            nc.vector.tensor_tensor(
                out=ohB[:],
                in0=tgtB[:, None, g * NBLK_G:(g + 1) * NBLK_G]
                .to_broadcast([128, 128, NBLK_G]),
                in1=iotaE[:],
                op=mybir.AluOpType.is_equal)

            for rr in range(RPG):
                r = g * RPG + rr
                ps = psagg.tile([128, 128], F32)
                for j in range(KA):
                    nc.tensor.matmul(ps[:], msgA[:, rr * KA + j, :],
                                     ohA[:, :, rr * KA + j],
                                     start=(j == 0), stop=False)
                for j in range(KA):
                    nc.tensor.matmul(ps[:], msgB[:, rr * KA + j, :],
                                     ohB[:, :, rr * KA + j],
                                     start=False, stop=(j == KA - 1))
                mean = pmean.tile([128, 128], BF16)
                nc.vector.tensor_mul(mean[:], ps[:],
                                     recip[:, r * 128:(r + 1) * 128])
                ps2 = pslin.tile([128, 128], F32)
                nc.tensor.matmul(ps2[:], Wl[:], mean[:], start=True, stop=False)
                nc.tensor.matmul(ps2[:], Wr[:], xT[:, r * 128:(r + 1) * 128],
                                 start=False, stop=True)
                if layer == 1:
                    hT = phT.tile([128, 128], BF16)
                    nc.scalar.activation(hT[:], ps2[:],
                                         mybir.ActivationFunctionType.Relu,
                                         bias=bv[:])
                    pst = pstr.tile([128, 128], BF16)
                    nc.tensor.transpose(pst[:], hT[:], ident[:])
                    nc.vector.tensor_copy(hstage[:, r, :], pst[:])
                else:
                    nc.scalar.activation(ostage[:, r * 128:(r + 1) * 128], ps2[:],
                                         mybir.ActivationFunctionType.Identity,
                                         bias=bv[:])
            # stream this group's outputs out now so the final write doesn't
            # serialize after the last gather
            if layer == 1:
                hview = hout.ap().rearrange("(t p) f -> p t f", p=128)
                nc.sync.dma_start(hview[:, g * RPG:(g + 1) * RPG, :],
                                  hstage[:, g * RPG:(g + 1) * RPG, :])
            else:
                nc.sync.dma_start(tout[:, g * RPG * 128:(g + 1) * RPG * 128],
                                  ostage[:, g * RPG * 128:(g + 1) * RPG * 128])
    nc.compile()
    return nc


def _wrap_idxs(stream, groups, GN):
    """[groups*GN] idx stream -> [128, groups*GN/16] int16 sbuf wrap layout,
    wrapped independently per gather call (per group)."""
    a = stream.reshape(groups, GN // 16, 16).transpose(0, 2, 1)  # [G,16,C]
    a = a.reshape(groups * 16, GN // 16)
    a = np.concatenate([a[g * 16:(g + 1) * 16] for g in range(groups)], axis=1)
    return np.tile(a, (8, 1)).astype(np.int16)


def _bin_nodes(deg):
    """Degree-balanced assignment of nodes to 320 bins of 128 slots.
    Snake-LPT: each round assigns the next 320 highest-degree nodes, largest
    degree to least-loaded bin. Returns pos_of_node [N] (bin*128 + slot)."""
    order = np.argsort(-deg, kind="stable")
    loads = np.zeros(NBINS, np.int64)
    pos_of_node = np.empty(N, np.int64)
    nrounds = (N + NBINS - 1) // NBINS
    for rnd in range(nrounds):
        chunk = order[rnd * NBINS:(rnd + 1) * NBINS]
        border = np.argsort(loads, kind="stable")[:len(chunk)]
        pos_of_node[chunk] = border * 128 + rnd
        loads[border] += deg[chunk]
    return pos_of_node, loads


def _core_streams(src_c, rloc_c, slot_c, KA):
    """Per-core A/B half assignment + idx/tgt stream construction.

    Returns (idx[2, RANGES, KA*128] int16 row ids, tgt[2, RANGES, KA*128]
    uint8 slot targets (255 = pad), assignment [N] in {-1,0,1},
    rowof [N] local row within half)."""
    CAP = KA * 128
    assignment = np.full(N, -1, np.int8)
    rowof = np.zeros(N, np.int32)
    nxt = [0, 0]
    idx = np.zeros((2, RANGES, CAP), np.int16)
    tgt = np.full((2, RANGES, CAP), 255, np.int16)

    order = np.argsort(rloc_c, kind="stable")
    src_s = src_c[order]
    slot_s = slot_c[order]
    bounds = np.searchsorted(rloc_c[order], np.arange(RANGES + 1))
    for r in range(RANGES):
        lo, hi = bounds[r], bounds[r + 1]
        if lo == hi:
            continue
        s = src_s[lo:hi]
        sl = slot_s[lo:hi]
        srcs_u, inv, cnts = np.unique(s, return_inverse=True,
                                      return_counts=True)
        known = assignment[srcs_u]
        nA = int(cnts[known == 0].sum())
        nB = int(cnts[known == 1].sum())
        new_i = np.where(known < 0)[0]
        heavy = new_i[cnts[new_i] >= 2]
        heavy = heavy[np.argsort(-cnts[heavy], kind="stable")]
        for i in heavy:
            h = 0 if nA <= nB else 1
            assignment[srcs_u[i]] = h
            rowof[srcs_u[i]] = nxt[h]
            nxt[h] += 1
            if h == 0:
                nA += int(cnts[i])
            else:
                nB += int(cnts[i])
        singles = new_i[cnts[new_i] == 1]
        S = len(singles)
        aS = min(max((S + nB - nA + 1) // 2, 0), S)
        sa, sb = singles[:aS], singles[aS:]
        assignment[srcs_u[sa]] = 0
        rowof[srcs_u[sa]] = nxt[0] + np.arange(aS)
        nxt[0] += aS
        nA += aS
        assignment[srcs_u[sb]] = 1
        rowof[srcs_u[sb]] = nxt[1] + np.arange(S - aS)
        nxt[1] += S - aS
        nB += S - aS
        if nA > CAP or nB > CAP:
            raise OverflowError(f"bin overflow nA={nA} nB={nB} cap={CAP}")
        half_e = assignment[s]
        rows_e = rowof[s]
        for h in (0, 1):
            m = half_e == h
            k = int(m.sum())
            idx[h, r, :k] = rows_e[m]
            tgt[h, r, :k] = sl[m]
    if nxt[0] > HALF_ROWS or nxt[1] > HALF_ROWS:
        raise OverflowError(f"half-table overflow {nxt}")
    return idx, tgt, assignment, rowof


def preprocess(x, edge_index, KA):
    src = np.asarray(edge_index[0], dtype=np.int64)
    dst = np.asarray(edge_index[1], dtype=np.int64)
    deg = np.bincount(dst, minlength=N)
    recip = (1.0 / np.maximum(deg, 1)).astype(np.float32)

    pos_of_node, loads = _bin_nodes(deg)
    if loads.max() > KA * 2 * 128:
        raise OverflowError(f"bin load {loads.max()} > {KA * 2 * 128}")
    node_of_pos = np.full(CORES * NPAD, -1, np.int64)
    node_of_pos[pos_of_node] = np.arange(N)

    dstpos = pos_of_node[dst]
    ecore = dstpos // NPAD
    erloc = (dstpos % NPAD) // 128
    eslot = dstpos % 128

    xv = np.asarray(x, dtype=np.float32)
    cores = []
    GN = RPG * KA * 128
    for c in range(CORES):
        m = ecore == c
        idx, tgt, assignment, rowof = _core_streams(
            src[m], erloc[m], eslot[m], KA)
        wrapA = _wrap_idxs(idx[0].reshape(-1), GROUPS, GN)
        wrapB = _wrap_idxs(idx[1].reshape(-1), GROUPS, GN)
        # tgt sbuf layout [128 partitions, RANGES*KA blocks]
        tgtA = np.ascontiguousarray(
            tgt[0].reshape(RANGES * KA, 128).T.astype(np.float32)).astype(NP_BF16)
        tgtB = np.ascontiguousarray(
            tgt[1].reshape(RANGES * KA, 128).T.astype(np.float32)).astype(NP_BF16)
        nodesA = np.where(assignment == 0)[0]
        nodesB = np.where(assignment == 1)[0]
        # rows of each half -> global node (-1 unused)
        rsrcA = np.full(HALF_ROWS, -1, np.int64)
        rsrcA[rowof[nodesA]] = nodesA
        rsrcB = np.full(HALF_ROWS, -1, np.int64)
        rsrcB[rowof[nodesB]] = nodesB
        rsrc = np.concatenate([rsrcA, rsrcB])
        own = node_of_pos[c * NPAD:(c + 1) * NPAD]
        cores.append(dict(wrapA=wrapA, wrapB=wrapB, tgtA=tgtA, tgtB=tgtB,
                          rsrc=rsrc, own=own))

    def table_from(feats_by_node):
        """feats_by_node: [N, D] float32 -> per-core permuted tables."""
        out = []
        for c in range(CORES):
            t = np.zeros((2 * HALF_ROWS, D), NP_BF16)
            rs = cores[c]["rsrc"]
            used = rs >= 0
            t[used] = feats_by_node[rs[used]].astype(NP_BF16)
            out.append(t)
        return out

    xT = []
    recipb = []
    for c in range(CORES):
        own = cores[c]["own"]
        used = own >= 0
        t = np.zeros((NPAD, D), np.float32)
        t[used] = xv[own[used]]
        xT.append(np.ascontiguousarray(t.T).astype(NP_BF16))
        rb = np.zeros((NPAD,), np.float32)
        rb[used] = recip[own[used]]
        recipb.append(np.broadcast_to(rb.astype(NP_BF16), (128, NPAD)).copy())

    return cores, table_from, xT, recipb, node_of_pos, xv


def kernel(x, edge_index, W1_l, b1, W1_r, W2_l, b2, W2_r, _timing=None):
    for KA in (8, 9, 10):
        try:
            cores, table_from, xT, recipb, node_of_pos, xv = preprocess(
                x, edge_index, KA)
            break
        except OverflowError:
            continue
    else:
        raise RuntimeError("binning failed")

    if KA not in _prog_cache:
        _prog_cache[KA] = (build_program(1, KA), build_program(2, KA))
    nc1, nc2 = _prog_cache[KA]

    def wmat(w):
        return np.asarray(w, dtype=np.float32).astype(NP_BF16)

    def bcol(b):
        return np.asarray(b, dtype=np.float32).reshape(128, 1)

    iota = np.ascontiguousarray(
        np.broadcast_to(np.arange(128, dtype=np.float32), (128, 128))
    ).astype(NP_BF16)
    tables1 = table_from(xv)
    maps1 = []
    for c in range(CORES):
        cc = cores[c]
        maps1.append(dict(table=tables1[c], idxA=cc["wrapA"], idxB=cc["wrapB"],
                          tgtA=cc["tgtA"], tgtB=cc["tgtB"], iota=iota,
                          xT=xT[c], recipb=recipb[c], Wl=wmat(W1_l),
                          Wr=wmat(W1_r), bvec=bcol(b1),
                          ident=np.eye(128, dtype=NP_BF16)))
    r1 = bass_utils.run_bass_kernel_spmd(nc1, maps1, core_ids=list(range(CORES)))

    # h by global node id (houts are in pos order)
    h_pos = np.concatenate([r1.results[c]["hout"] for c in range(CORES)], axis=0)
    h_node = np.zeros((N, D), np.float32)
    valid = node_of_pos >= 0
    h_node[node_of_pos[valid]] = h_pos[valid]
    tables2 = table_from(h_node)

    maps2 = []
    for c in range(CORES):
        cc = cores[c]
        hT_own = np.ascontiguousarray(r1.results[c]["hout"].T)
        maps2.append(dict(table=tables2[c], idxA=cc["wrapA"], idxB=cc["wrapB"],
                          tgtA=cc["tgtA"], tgtB=cc["tgtB"], iota=iota,
                          xT=hT_own, recipb=recipb[c], Wl=wmat(W2_l),
                          Wr=wmat(W2_r), bvec=bcol(b2)))
    r2 = bass_utils.run_bass_kernel_spmd(nc2, maps2, core_ids=list(range(CORES)))
    if _timing is not None:
        _timing["nc1"] = nc1
        _timing["nc2"] = nc2

    out = np.empty((N, D), np.float32)
    for c in range(CORES):
        own = cores[c]["own"]
        used = own >= 0
        out[own[used]] = r2.results[c]["tout"].T[used]
    return out


# revision 21
# speedup vs baseline: 1.1868x; 1.1868x over previous
"""Two-layer SAGEConv (mean aggregation) GNN on 8 trn2 NeuronCores.

Strategy (dst-sharded graph parallel, v4 "paired gather"):
  - dst nodes are packed into 320 bins of 128 slots (40 bins per core) by a
    degree-balanced snake-LPT, so every bin holds <= 2048 edges. Each bin is
    one psum "range".
  - Each core gets its OWN permuted pair-table [20480, 256] bf16: row k holds
    features of TWO source nodes (A-half cols 0:128, B-half cols 128:256).
    One 512-byte gather descriptor therefore serves up to two edges: sources
    that first co-occur in the same bin are paired at the same row, and each
    128-edge slot can route its A-column value and its B-column value to
    independent dst slots. This cuts gather descriptors ~20% below one-per-
    edge, and 512B descriptors avoid the small-transfer DMA penalty.
  - Per-bin slot counts are uneven, so bins are relabeled per core in
    descending slot count and the program is compiled for the shared
    per-range block-count profile (max across cores).
  - Routing matrices are generated ON-CHIP from slot-target bytes via
    broadcast is_equal on DVE in a slot-major layout (keeps last-dim
    stride-1 so the DVE 2x perf mode applies).
  - Aggregation per range: psum[f, slot] += sum_j msg[:, j, 0:128].T @ RA_j
    + msg[:, j, 128:256].T @ RB_j; then mean scale (1/deg along free dim),
    hT = act(W_l.T @ meanT + W_r.T @ xT + b). Layer 1 PE-transposes hT back
    to row-major to rebuild the h table (host assembles between launches).
"""
import numpy as np
import ml_dtypes
from contextlib import ExitStack

import concourse.bass as bass
import concourse.mybir as mybir
import concourse.tile as tile
from concourse import bacc
from concourse.library_config import mlp
from concourse import bass_utils

BF16 = mybir.dt.bfloat16
F32 = mybir.dt.float32
I16 = mybir.dt.int16
NP_BF16 = ml_dtypes.bfloat16

N = 40000
D = 128
CORES = 8
RANGES = 40            # bins (dst ranges of 128 slots) per core
NPAD = RANGES * 128    # 5120 dst positions per core
NBINS = CORES * RANGES
PAIR_ROWS = 20480      # pair-table rows (int16-indexable)
GMAX = 40              # max message blocks per gather group (SBUF budget)

_prog_cache = {}


def _make_groups(KP):
    """Split program ranges into gather groups of <= GMAX blocks.
    Returns list of (range_lo, range_hi, block_offset, nblk). The final
    group is kept small (<= 2 ranges) to shorten the post-gather drain."""
    groups = []
    lo = 0
    off = 0
    cur = 0
    for r in range(RANGES - 2):
        if cur + KP[r] > GMAX and cur > 0:
            groups.append((lo, r, off, cur))
            off += cur
            lo = r
            cur = 0
        cur += KP[r]
    groups.append((lo, RANGES - 2, off, cur))
    off += cur
    tail = KP[RANGES - 2] + KP[RANGES - 1]
    groups.append((RANGES - 2, RANGES, off, tail))
    return groups


def build_program(layer, KP):
    """One SPMD program for one SAGEConv layer. KP[r] = pair-blocks of range r."""
    KP = list(KP)
    TOTBLK = sum(KP)
    IDX_COLS = TOTBLK * 8          # idx wrap cols (num_idxs/16 per call, concat)
    groups = _make_groups(KP)
    starts = np.concatenate([[0], np.cumsum(KP)]).astype(int)

    nc = bacc.Bacc("TRN2", target_bir_lowering=False, debug=False)
    table = nc.dram_tensor("table", [PAIR_ROWS, 2 * D], BF16, kind="ExternalInput")
    idx_d = nc.dram_tensor("idxs", [128, IDX_COLS], I16, kind="ExternalInput")
    tgtA_d = nc.dram_tensor("tgtA", [128, TOTBLK], BF16, kind="ExternalInput")
    tgtB_d = nc.dram_tensor("tgtB", [128, TOTBLK], BF16, kind="ExternalInput")
    iota_d = nc.dram_tensor("iota", [128, 128], BF16, kind="ExternalInput")
    xT_d = nc.dram_tensor("xT", [128, NPAD], BF16, kind="ExternalInput")
    recip_d = nc.dram_tensor("recipb", [128, NPAD], BF16, kind="ExternalInput")
    Wl_d = nc.dram_tensor("Wl", [128, 128], BF16, kind="ExternalInput")
    Wr_d = nc.dram_tensor("Wr", [128, 128], BF16, kind="ExternalInput")
    b_d = nc.dram_tensor("bvec", [128, 1], F32, kind="ExternalInput")
    # feature-major [f, pos]: host transposes (it re-permutes tables anyway)
    tout = nc.dram_tensor("tout", [128, NPAD], BF16, kind="ExternalOutput")

    with tile.TileContext(nc) as tc, ExitStack() as ctx:
        const = ctx.enter_context(tc.tile_pool(name="const", bufs=1))
        pmsg = ctx.enter_context(tc.tile_pool(name="msg", bufs=3))
        pohA = ctx.enter_context(tc.tile_pool(name="ohA", bufs=2))
        pohB = ctx.enter_context(tc.tile_pool(name="ohB", bufs=2))
        psagg = ctx.enter_context(tc.tile_pool(name="psagg", bufs=4, space="PSUM"))
        pslin = ctx.enter_context(tc.tile_pool(name="pslin", bufs=2, space="PSUM"))
        pmean = ctx.enter_context(tc.tile_pool(name="mean", bufs=3))

        nc.gpsimd.load_library(mlp)

        idxs = const.tile([128, IDX_COLS], I16)
        nc.sync.dma_start(idxs[:], idx_d[:])
        tgtA = const.tile([128, TOTBLK], BF16)
        nc.sync.dma_start(tgtA[:], tgtA_d[:])
        tgtB = const.tile([128, TOTBLK], BF16)
        nc.sync.dma_start(tgtB[:], tgtB_d[:])
        iota = const.tile([128, 128], BF16)
        nc.sync.dma_start(iota[:], iota_d[:])
        # iotaE[p, f, b] = f — stride-1 last dims for the DVE 2x perf mode
        iotaE = const.tile([128, 128, GMAX], BF16)
        nc.vector.tensor_copy(iotaE[:],
                              iota[:, :, None].to_broadcast([128, 128, GMAX]))
        xT = const.tile([128, NPAD], BF16)
        nc.sync.dma_start(xT[:], xT_d[:])
        recip = const.tile([128, NPAD], BF16)
        nc.sync.dma_start(recip[:], recip_d[:])
        Wl = const.tile([128, 128], BF16)
        nc.sync.dma_start(Wl[:], Wl_d[:])
        Wr = const.tile([128, 128], BF16)
        nc.sync.dma_start(Wr[:], Wr_d[:])
        bv = const.tile([128, 1], F32)
        nc.sync.dma_start(bv[:], b_d[:])
        ostage = const.tile([128, NPAD], BF16)

        def gen_oh(pool_or_const, tgt, boff, nblk):
            oh = pool_or_const.tile([128, 128, nblk], BF16)
            nc.vector.tensor_tensor(
                out=oh[:],
                in0=tgt[:, None, boff:boff + nblk]
                .to_broadcast([128, 128, nblk]),
                in1=iotaE[:, :, :nblk],
                op=mybir.AluOpType.is_equal)
            return oh

        # prefetch the last two groups' routing matrices so the post-gather
        # drain isn't serialized behind their DVE generation
        pre = {}
        for gi in (len(groups) - 2, len(groups) - 1):
            _, _, boff, nblk = groups[gi]
            pre[gi] = (gen_oh(const, tgtA, boff, nblk),
                       gen_oh(const, tgtB, boff, nblk))

        for gi, (rlo, rhi, boff, nblk) in enumerate(groups):
            GN = nblk * 128
            msg = pmsg.tile([128, nblk, 2 * D], BF16)
            nc.gpsimd.dma_gather(msg[:], table[:, :],
                                 idxs[:, boff * 8:(boff + nblk) * 8],
                                 GN, GN, 2 * D, single_packet=False)
            if gi in pre:
                ohA, ohB = pre[gi]
            else:
                ohA = gen_oh(pohA, tgtA, boff, nblk)
                ohB = gen_oh(pohB, tgtB, boff, nblk)

            for r in range(rlo, rhi):
                k = KP[r]
                ps = psagg.tile([128, 128], F32)
                for j in range(k):
                    bb = starts[r] - boff + j
                    nc.tensor.matmul(ps[:], msg[:, bb, 0:D],
                                     ohA[:, :, bb],
                                     start=(j == 0), stop=False)
                    nc.tensor.matmul(ps[:], msg[:, bb, D:2 * D],
                                     ohB[:, :, bb],
                                     start=False, stop=(j == k - 1))
                mean = pmean.tile([128, 128], BF16)
                nc.vector.tensor_mul(mean[:], ps[:],
                                     recip[:, r * 128:(r + 1) * 128])
                ps2 = pslin.tile([128, 128], F32)
                nc.tensor.matmul(ps2[:], Wl[:], mean[:], start=True, stop=False)
                nc.tensor.matmul(ps2[:], Wr[:], xT[:, r * 128:(r + 1) * 128],
                                 start=False, stop=True)
                nc.scalar.activation(
                    ostage[:, r * 128:(r + 1) * 128], ps2[:],
                    mybir.ActivationFunctionType.Relu if layer == 1
                    else mybir.ActivationFunctionType.Identity,
                    bias=bv[:])
            # stream this group's outputs now so the final write doesn't
            # serialize after the last gather
            nc.sync.dma_start(tout[:, rlo * 128:rhi * 128],
                              ostage[:, rlo * 128:rhi * 128])
    nc.compile()
    return nc


def _wrap_idxs(streams):
    """list of per-call idx streams (len % 2048 == 0) -> [128, sum/16] int16
    sbuf wrap layout (16-partition wrap per call, replicated to 128)."""
    cols = []
    for s in streams:
        a = s.reshape(-1, 16).T  # [16, GN/16]
        cols.append(a)
    a = np.concatenate(cols, axis=1)
    return np.tile(a, (8, 1)).astype(np.int16)


def _bin_nodes(deg):
    """Degree-balanced assignment of nodes to 320 bins of 128 slots."""
    order = np.argsort(-deg, kind="stable")
    loads = np.zeros(NBINS, np.int64)
    bin_of_node = np.empty(N, np.int64)
    slot_of_node = np.empty(N, np.int64)
    nrounds = (N + NBINS - 1) // NBINS
    for rnd in range(nrounds):
        chunk = order[rnd * NBINS:(rnd + 1) * NBINS]
        border = np.argsort(loads, kind="stable")[:len(chunk)]
        bin_of_node[chunk] = border
        slot_of_node[chunk] = rnd
        loads[border] += deg[chunk]
    return bin_of_node, slot_of_node, loads


def _ranks(rows):
    """Per-element rank within equal-value group of sorted-by-value `rows`,
    plus unique values and counts. rows need not be sorted."""
    o = np.argsort(rows, kind="stable")
    sr = rows[o]
    if len(sr) == 0:
        return np.empty(0, np.int64), np.empty(0, np.int64), np.empty(0, np.int64)
    newgrp = np.r_[True, sr[1:] != sr[:-1]]
    starts = np.flatnonzero(newgrp)
    grp = np.cumsum(newgrp) - 1
    pos = np.arange(len(sr)) - starts[grp]
    rank = np.empty(len(rows), np.int64)
    rank[o] = pos
    ur = sr[starts]
    cnt = np.diff(np.r_[starts, len(sr)])
    return rank, ur, cnt


def _core_streams(src_c, rloc_c, slot_c):
    """Per-core pair assignment + per-bin slot streams.

    Returns (streams: list of (idx_r, tgtA_r, tgtB_r) per physical bin,
    nslots [RANGES], assignment [N] in {-1,0,1}, rowof [N])."""
    assignment = np.full(N, -1, np.int8)
    rowof = np.zeros(N, np.int32)
    nxt = 0
    streams = []
    nslots = np.zeros(RANGES, np.int64)

    order = np.argsort(rloc_c, kind="stable")
    src_s = src_c[order]
    slot_s = slot_c[order]
    bounds = np.searchsorted(rloc_c[order], np.arange(RANGES + 1))
    for r in range(RANGES):
        lo, hi = bounds[r], bounds[r + 1]
        s = src_s[lo:hi]
        sl = slot_s[lo:hi]
        srcs_u, cnts = np.unique(s, return_counts=True)
        new_i = np.where(assignment[srcs_u] < 0)[0]
        # pair new sources (by descending multiplicity) at the same row:
        # element 2i -> A-half row nxt+i, element 2i+1 -> B-half row nxt+i
        new_sorted = new_i[np.argsort(-cnts[new_i], kind="stable")]
        a_i, b_i = new_sorted[0::2], new_sorted[1::2]
        npairs = len(a_i)
        assignment[srcs_u[a_i]] = 0
        rowof[srcs_u[a_i]] = nxt + np.arange(npairs)
        assignment[srcs_u[b_i]] = 1
        rowof[srcs_u[b_i]] = nxt + np.arange(len(b_i))
        nxt += npairs

        half_e = assignment[s]
        rows_e = rowof[s].astype(np.int64)
        mA = half_e == 0
        mB = ~mA
        rankA, urA, cntA = _ranks(rows_e[mA])
        rankB, urB, cntB = _ranks(rows_e[mB])
        ur = np.union1d(urA, urB)
        cA = np.zeros(len(ur), np.int64)
        cA[np.searchsorted(ur, urA)] = cntA
        cB = np.zeros(len(ur), np.int64)
        cB[np.searchsorted(ur, urB)] = cntB
        w = np.maximum(cA, cB)
        base = np.r_[0, np.cumsum(w)[:-1]]
        ns = int(w.sum())
        idx_r = np.repeat(ur, w).astype(np.int16)
        tgtA_r = np.full(ns, 255, np.int16)
        tgtA_r[base[np.searchsorted(ur, rows_e[mA])] + rankA] = sl[mA]
        tgtB_r = np.full(ns, 255, np.int16)
        tgtB_r[base[np.searchsorted(ur, rows_e[mB])] + rankB] = sl[mB]
        streams.append((idx_r, tgtA_r, tgtB_r))
        nslots[r] = ns
    if nxt > PAIR_ROWS:
        raise OverflowError(f"pair rows overflow {nxt}")
    return streams, nslots, assignment, rowof


def preprocess(x, edge_index):
    src = np.asarray(edge_index[0], dtype=np.int64)
    dst = np.asarray(edge_index[1], dtype=np.int64)
    deg = np.bincount(dst, minlength=N)
    recip = (1.0 / np.maximum(deg, 1)).astype(np.float32)

    bin_of_node, slot_of_node, loads = _bin_nodes(deg)
    ecore = bin_of_node[dst] // RANGES
    erloc = bin_of_node[dst] % RANGES
    eslot = slot_of_node[dst]

    xv = np.asarray(x, dtype=np.float32)
    per_core = []
    nslots_all = np.zeros((CORES, RANGES), np.int64)
    for c in range(CORES):
        m = ecore == c
        streams, nslots, assignment, rowof = _core_streams(
            src[m], erloc[m], eslot[m])
        per_core.append((streams, nslots, assignment, rowof))
        nslots_all[c] = nslots

    # per-core bin relabel (desc slot count) + shared block-count profile
    perms = [np.argsort(-nslots_all[c], kind="stable") for c in range(CORES)]
    sorted_ns = np.stack([nslots_all[c][perms[c]] for c in range(CORES)])
    profile = sorted_ns.max(axis=0)
    KP = np.maximum(np.ceil(profile / 128).astype(int), 1)
    if profile.max() > 2048:
        raise OverflowError(f"range overflow {profile.max()}")
    TOTBLK = int(KP.sum())
    groups = _make_groups(list(KP))

    cores = []
    for c in range(CORES):
        streams, nslots, assignment, rowof = per_core[c]
        perm = perms[c]
        idx_full = np.zeros((TOTBLK * 128,), np.int16)
        tgtA_full = np.full((TOTBLK * 128,), 255, np.int16)
        tgtB_full = np.full((TOTBLK * 128,), 255, np.int16)
        off = 0
        for r in range(RANGES):
            idx_r, tgtA_r, tgtB_r = streams[perm[r]]
            ns = len(idx_r)
            idx_full[off:off + ns] = idx_r
            tgtA_full[off:off + ns] = tgtA_r
            tgtB_full[off:off + ns] = tgtB_r
            off += KP[r] * 128
        call_streams = [idx_full[boff * 128:(boff + nblk) * 128]
                        for (_, _, boff, nblk) in groups]
        wrap = _wrap_idxs(call_streams)
        tgtA = np.ascontiguousarray(
            tgtA_full.reshape(TOTBLK, 128).T.astype(np.float32)).astype(NP_BF16)
        tgtB = np.ascontiguousarray(
            tgtB_full.reshape(TOTBLK, 128).T.astype(np.float32)).astype(NP_BF16)

        nodesA = np.where(assignment == 0)[0]
        nodesB = np.where(assignment == 1)[0]
        rsrcA = np.full(PAIR_ROWS, -1, np.int64)
        rsrcA[rowof[nodesA]] = nodesA
        rsrcB = np.full(PAIR_ROWS, -1, np.int64)
        rsrcB[rowof[nodesB]] = nodesB
        # own nodes in relabeled pos order
        own = np.full(NPAD, -1, np.int64)
        mc = bin_of_node // RANGES == c
        nodes_c = np.where(mc)[0]
        rinv = np.empty(RANGES, np.int64)
        rinv[perm] = np.arange(RANGES)
        own[rinv[bin_of_node[nodes_c] % RANGES] * 128
            + slot_of_node[nodes_c]] = nodes_c
        cores.append(dict(wrap=wrap, tgtA=tgtA, tgtB=tgtB,
                          rsrcA=rsrcA, rsrcB=rsrcB, own=own))

    def table_from(feats_by_node):
        out = []
        for c in range(CORES):
            t = np.zeros((PAIR_ROWS, 2 * D), NP_BF16)
            for half, key in ((0, "rsrcA"), (1, "rsrcB")):
                rs = cores[c][key]
                used = rs >= 0
                t[used, half * D:(half + 1) * D] = \
                    feats_by_node[rs[used]].astype(NP_BF16)
            out.append(t)
        return out

    xT = []
    recipb = []
    for c in range(CORES):
        own = cores[c]["own"]
        used = own >= 0
        t = np.zeros((NPAD, D), np.float32)
        t[used] = xv[own[used]]
        xT.append(np.ascontiguousarray(t.T).astype(NP_BF16))
        rb = np.zeros((NPAD,), np.float32)
        rb[used] = recip[own[used]]
        recipb.append(np.broadcast_to(rb.astype(NP_BF16), (128, NPAD)).copy())

    return cores, table_from, xT, recipb, tuple(KP.tolist()), xv


def kernel(x, edge_index, W1_l, b1, W1_r, W2_l, b2, W2_r, _timing=None):
    cores, table_from, xT, recipb, KP, xv = preprocess(x, edge_index)

    if KP not in _prog_cache:
        _prog_cache[KP] = (build_program(1, KP), build_program(2, KP))
    nc1, nc2 = _prog_cache[KP]

    def wmat(w):
        return np.asarray(w, dtype=np.float32).astype(NP_BF16)

    def bcol(b):
        return np.asarray(b, dtype=np.float32).reshape(128, 1)

    iota = np.ascontiguousarray(
        np.broadcast_to(np.arange(128, dtype=np.float32), (128, 128))
    ).astype(NP_BF16)
    tables1 = table_from(xv)
    maps1 = []
    for c in range(CORES):
        cc = cores[c]
        maps1.append(dict(table=tables1[c], idxs=cc["wrap"],
                          tgtA=cc["tgtA"], tgtB=cc["tgtB"], iota=iota,
                          xT=xT[c], recipb=recipb[c], Wl=wmat(W1_l),
                          Wr=wmat(W1_r), bvec=bcol(b1)))
    r1 = bass_utils.run_bass_kernel_spmd(nc1, maps1, core_ids=list(range(CORES)))

    # h by global node id (houts are feature-major in relabeled pos order)
    h_node = np.zeros((N, D), np.float32)
    for c in range(CORES):
        own = cores[c]["own"]
        used = own >= 0
        h_node[own[used]] = r1.results[c]["tout"].T[used]
    tables2 = table_from(h_node)

    maps2 = []
    for c in range(CORES):
        cc = cores[c]
        hT_own = np.asarray(r1.results[c]["tout"], dtype=np.float32).astype(NP_BF16)
        maps2.append(dict(table=tables2[c], idxs=cc["wrap"],
                          tgtA=cc["tgtA"], tgtB=cc["tgtB"], iota=iota,
                          xT=hT_own, recipb=recipb[c], Wl=wmat(W2_l),
                          Wr=wmat(W2_r), bvec=bcol(b2)))
    r2 = bass_utils.run_bass_kernel_spmd(nc2, maps2, core_ids=list(range(CORES)))
    if _timing is not None:
        _timing["nc1"] = nc1
        _timing["nc2"] = nc2

    out = np.empty((N, D), np.float32)
    for c in range(CORES):
        own = cores[c]["own"]
        used = own >= 0
        out[own[used]] = r2.results[c]["tout"].T[used]
    return out


# revision 26
# speedup vs baseline: 1.3453x; 1.1335x over previous
"""Two-layer SAGEConv (mean aggregation) GNN on 8 trn2 NeuronCores.

Strategy (dst-sharded graph parallel, v4 "paired gather"):
  - dst nodes are packed into 320 bins of 128 slots (40 bins per core) by a
    degree-balanced snake-LPT, so every bin holds <= 2048 edges. Each bin is
    one psum "range".
  - Each core gets its OWN permuted pair-table [20480, 256] bf16: row k holds
    features of TWO source nodes (A-half cols 0:128, B-half cols 128:256).
    One 512-byte gather descriptor therefore serves up to two edges: sources
    that first co-occur in the same bin are paired at the same row, and each
    128-edge slot can route its A-column value and its B-column value to
    independent dst slots. This cuts gather descriptors ~20% below one-per-
    edge, and 512B descriptors avoid the small-transfer DMA penalty.
  - Per-bin slot counts are uneven, so bins are relabeled per core in
    descending slot count and the program is compiled for the shared
    per-range block-count profile (max across cores).
  - Routing matrices are generated ON-CHIP from slot-target bytes via
    broadcast is_equal on DVE in a slot-major layout (keeps last-dim
    stride-1 so the DVE 2x perf mode applies).
  - Aggregation per range: psum[f, slot] += sum_j msg[:, j, 0:128].T @ RA_j
    + msg[:, j, 128:256].T @ RB_j; then mean scale (1/deg along free dim),
    hT = act(W_l.T @ meanT + W_r.T @ xT + b). Layer 1 PE-transposes hT back
    to row-major to rebuild the h table (host assembles between launches).
"""
import numpy as np
import ml_dtypes
from contextlib import ExitStack

import concourse.bass as bass
import concourse.mybir as mybir
import concourse.tile as tile
from concourse import bacc
from concourse.library_config import mlp
from concourse import bass_utils

BF16 = mybir.dt.bfloat16
F32 = mybir.dt.float32
I16 = mybir.dt.int16
NP_BF16 = ml_dtypes.bfloat16

N = 40000
D = 128
CORES = 8
RANGES = 40            # bins (dst ranges of 128 slots) per core
NPAD = RANGES * 128    # 5120 dst positions per core
NBINS = CORES * RANGES
PAIR_ROWS = 20480      # pair-table rows (int16-indexable)
GMAX = 40              # max message blocks per gather group (SBUF budget)

_prog_cache = {}


def _make_groups(KP):
    """Split program ranges into gather groups of <= GMAX blocks.
    Returns list of (range_lo, range_hi, block_offset, nblk). The final
    group is kept small (<= 2 ranges) to shorten the post-gather drain."""
    groups = []
    lo = 0
    off = 0
    cur = 0
    for r in range(RANGES - 2):
        if cur + KP[r] > GMAX and cur > 0:
            groups.append((lo, r, off, cur))
            off += cur
            lo = r
            cur = 0
        cur += KP[r]
    groups.append((lo, RANGES - 2, off, cur))
    off += cur
    tail = KP[RANGES - 2] + KP[RANGES - 1]
    groups.append((RANGES - 2, RANGES, off, tail))
    return groups


def build_program(layer, KP):
    """One SPMD program for one SAGEConv layer. KP[r] = pair-blocks of range r."""
    KP = list(KP)
    TOTBLK = sum(KP)
    IDX_COLS = TOTBLK * 8          # idx wrap cols (num_idxs/16 per call, concat)
    groups = _make_groups(KP)
    starts = np.concatenate([[0], np.cumsum(KP)]).astype(int)

    nc = bacc.Bacc("TRN2", target_bir_lowering=False, debug=False)
    table = nc.dram_tensor("table", [PAIR_ROWS, 2 * D], BF16, kind="ExternalInput")
    idx_d = nc.dram_tensor("idxs", [128, IDX_COLS], I16, kind="ExternalInput")
    tgtA_d = nc.dram_tensor("tgtA", [128, TOTBLK], BF16, kind="ExternalInput")
    tgtB_d = nc.dram_tensor("tgtB", [128, TOTBLK], BF16, kind="ExternalInput")
    iota_d = nc.dram_tensor("iota", [128, 128], BF16, kind="ExternalInput")
    xT_d = nc.dram_tensor("xT", [128, NPAD], BF16, kind="ExternalInput")
    recip_d = nc.dram_tensor("recipb", [128, NPAD], BF16, kind="ExternalInput")
    Wl_d = nc.dram_tensor("Wl", [128, 128], BF16, kind="ExternalInput")
    Wr_d = nc.dram_tensor("Wr", [128, 128], BF16, kind="ExternalInput")
    b_d = nc.dram_tensor("bvec", [128, 1], F32, kind="ExternalInput")
    # feature-major [f, pos]: host transposes (it re-permutes tables anyway)
    tout = nc.dram_tensor("tout", [128, NPAD], BF16, kind="ExternalOutput")

    with tile.TileContext(nc) as tc, ExitStack() as ctx:
        const = ctx.enter_context(tc.tile_pool(name="const", bufs=1))
        pmsg = ctx.enter_context(tc.tile_pool(name="msg", bufs=3))
        pohA = ctx.enter_context(tc.tile_pool(name="ohA", bufs=2))
        pohB = ctx.enter_context(tc.tile_pool(name="ohB", bufs=2))
        psagg = ctx.enter_context(tc.tile_pool(name="psagg", bufs=4, space="PSUM"))
        pslin = ctx.enter_context(tc.tile_pool(name="pslin", bufs=2, space="PSUM"))
        pmean = ctx.enter_context(tc.tile_pool(name="mean", bufs=3))

        nc.gpsimd.load_library(mlp)

        idxs = const.tile([128, IDX_COLS], I16)
        nc.sync.dma_start(idxs[:], idx_d[:])
        tgtA = const.tile([128, TOTBLK], BF16)
        nc.sync.dma_start(tgtA[:], tgtA_d[:])
        tgtB = const.tile([128, TOTBLK], BF16)
        nc.sync.dma_start(tgtB[:], tgtB_d[:])
        iota = const.tile([128, 128], BF16)
        nc.sync.dma_start(iota[:], iota_d[:])
        # iotaE[p, f, b] = f — stride-1 last dims for the DVE 2x perf mode
        iotaE = const.tile([128, 128, GMAX], BF16)
        nc.vector.tensor_copy(iotaE[:],
                              iota[:, :, None].to_broadcast([128, 128, GMAX]))
        xT = const.tile([128, NPAD], BF16)
        nc.sync.dma_start(xT[:], xT_d[:])
        recip = const.tile([128, NPAD], BF16)
        nc.sync.dma_start(recip[:], recip_d[:])
        Wl = const.tile([128, 128], BF16)
        nc.sync.dma_start(Wl[:], Wl_d[:])
        Wr = const.tile([128, 128], BF16)
        nc.sync.dma_start(Wr[:], Wr_d[:])
        bv = const.tile([128, 1], F32)
        nc.sync.dma_start(bv[:], b_d[:])
        ostage = const.tile([128, NPAD], BF16)

        def gen_oh(pool_or_const, tgt, boff, nblk):
            oh = pool_or_const.tile([128, 128, nblk], BF16)
            nc.vector.tensor_tensor(
                out=oh[:],
                in0=tgt[:, None, boff:boff + nblk]
                .to_broadcast([128, 128, nblk]),
                in1=iotaE[:, :, :nblk],
                op=mybir.AluOpType.is_equal)
            return oh

        # DVE executes in issue order: generate routing matrices one group
        # AHEAD of the compute that consumes the previous group's psums, so
        # gen(g+1) is not head-of-line blocked behind the means of group g.
        oh_next = (gen_oh(pohA, tgtA, groups[0][2], groups[0][3]),
                   gen_oh(pohB, tgtB, groups[0][2], groups[0][3]))
        for gi, (rlo, rhi, boff, nblk) in enumerate(groups):
            GN = nblk * 128
            msg = pmsg.tile([128, nblk, 2 * D], BF16)
            nc.gpsimd.dma_gather(msg[:], table[:, :],
                                 idxs[:, boff * 8:(boff + nblk) * 8],
                                 GN, GN, 2 * D, single_packet=False)
            ohA, ohB = oh_next
            if gi + 1 < len(groups):
                nb = groups[gi + 1]
                oh_next = (gen_oh(pohA, tgtA, nb[2], nb[3]),
                           gen_oh(pohB, tgtB, nb[2], nb[3]))

            for r in range(rlo, rhi):
                k = KP[r]
                ps = psagg.tile([128, 128], F32)
                for j in range(k):
                    bb = starts[r] - boff + j
                    nc.tensor.matmul(ps[:], msg[:, bb, 0:D],
                                     ohA[:, :, bb],
                                     start=(j == 0), stop=False)
                    nc.tensor.matmul(ps[:], msg[:, bb, D:2 * D],
                                     ohB[:, :, bb],
                                     start=False, stop=(j == k - 1))
                mean = pmean.tile([128, 128], BF16)
                nc.vector.tensor_mul(mean[:], ps[:],
                                     recip[:, r * 128:(r + 1) * 128])
                ps2 = pslin.tile([128, 128], F32)
                nc.tensor.matmul(ps2[:], Wl[:], mean[:], start=True, stop=False)
                nc.tensor.matmul(ps2[:], Wr[:], xT[:, r * 128:(r + 1) * 128],
                                 start=False, stop=True)
                nc.scalar.activation(
                    ostage[:, r * 128:(r + 1) * 128], ps2[:],
                    mybir.ActivationFunctionType.Relu if layer == 1
                    else mybir.ActivationFunctionType.Identity,
                    bias=bv[:])
            # stream this group's outputs now so the final write doesn't
            # serialize after the last gather
            nc.sync.dma_start(tout[:, rlo * 128:rhi * 128],
                              ostage[:, rlo * 128:rhi * 128])
    nc.compile()
    return nc


def _wrap_idxs(streams):
    """list of per-call idx streams (len % 2048 == 0) -> [128, sum/16] int16
    sbuf wrap layout (16-partition wrap per call, replicated to 128)."""
    cols = []
    for s in streams:
        a = s.reshape(-1, 16).T  # [16, GN/16]
        cols.append(a)
    a = np.concatenate(cols, axis=1)
    return np.tile(a, (8, 1)).astype(np.int16)


def _bin_nodes(deg):
    """Degree-balanced assignment of nodes to 320 bins of 128 slots."""
    order = np.argsort(-deg, kind="stable")
    loads = np.zeros(NBINS, np.int64)
    bin_of_node = np.empty(N, np.int64)
    slot_of_node = np.empty(N, np.int64)
    nrounds = (N + NBINS - 1) // NBINS
    for rnd in range(nrounds):
        chunk = order[rnd * NBINS:(rnd + 1) * NBINS]
        border = np.argsort(loads, kind="stable")[:len(chunk)]
        bin_of_node[chunk] = border
        slot_of_node[chunk] = rnd
        loads[border] += deg[chunk]
    return bin_of_node, slot_of_node, loads


def _ranks(rows):
    """Per-element rank within equal-value group of sorted-by-value `rows`,
    plus unique values and counts. rows need not be sorted."""
    o = np.argsort(rows, kind="stable")
    sr = rows[o]
    if len(sr) == 0:
        return np.empty(0, np.int64), np.empty(0, np.int64), np.empty(0, np.int64)
    newgrp = np.r_[True, sr[1:] != sr[:-1]]
    starts = np.flatnonzero(newgrp)
    grp = np.cumsum(newgrp) - 1
    pos = np.arange(len(sr)) - starts[grp]
    rank = np.empty(len(rows), np.int64)
    rank[o] = pos
    ur = sr[starts]
    cnt = np.diff(np.r_[starts, len(sr)])
    return rank, ur, cnt


def _greedy_pair(keys):
    """Pair elements (indices) having equal keys: returns (a_idx, b_idx,
    leftover_idx). Elements are paired consecutively within equal-key runs."""
    o = np.argsort(keys, kind="stable")
    ks = keys[o]
    if len(ks) == 0:
        z = np.empty(0, np.int64)
        return z, z, z
    newg = np.r_[True, ks[1:] != ks[:-1]]
    starts = np.flatnonzero(newg)
    gid = np.cumsum(newg) - 1
    pos = np.arange(len(ks)) - starts[gid]
    sizes = np.diff(np.r_[starts, len(ks)])
    odd_last = (pos == sizes[gid] - 1) & (sizes[gid] % 2 == 1)
    paired = ~odd_last
    po = o[paired]
    return po[0::2], po[1::2], o[odd_last]


def _pair_sources(src_c, rloc_c):
    """Global pairing of this core's sources by their first-two-bins key:
    a pair sharing two bins saves a gather slot in both. Returns
    (assignment [N] in {-1,0,1}, rowof [N], rows_used)."""
    key = src_c * 64 + rloc_c
    ub = np.unique(key)
    usrc, ubin = ub >> 6, ub & 63
    first = np.r_[True, usrc[1:] != usrc[:-1]]
    idx_first = np.flatnonzero(first)
    srcs_u = usrc[idx_first]
    b1 = ubin[idx_first]
    nxt_is_same = np.r_[idx_first[1:] - idx_first[:-1] > 1,
                        len(ub) - idx_first[-1] > 1]
    b2 = np.where(nxt_is_same, ubin[np.minimum(idx_first + 1, len(ub) - 1)], 64)
    # round 1: match on (first bin, second bin); round 2: leftovers on b1
    a1, bb1, left = _greedy_pair(b1 * 65 + b2)
    a2, bb2, left2 = _greedy_pair(b1[left])
    a_i = np.concatenate([a1, left[a2]])
    b_i = np.concatenate([bb1, left[bb2]])
    single = left[left2]

    assignment = np.full(N, -1, np.int8)
    rowof = np.zeros(N, np.int32)
    npairs = len(a_i)
    assignment[srcs_u[a_i]] = 0
    rowof[srcs_u[a_i]] = np.arange(npairs)
    assignment[srcs_u[b_i]] = 1
    rowof[srcs_u[b_i]] = np.arange(len(b_i))
    assignment[srcs_u[single]] = 0
    rowof[srcs_u[single]] = npairs + np.arange(len(single))
    return assignment, rowof, npairs + len(single)


def _core_streams(src_c, rloc_c, slot_c):
    """Per-core pair assignment + per-bin slot streams.

    Returns (streams: list of (idx_r, tgtA_r, tgtB_r) per physical bin,
    nslots [RANGES], assignment [N] in {-1,0,1}, rowof [N])."""
    assignment, rowof, rows_used = _pair_sources(src_c, rloc_c)
    if rows_used > PAIR_ROWS:
        raise OverflowError(f"pair rows overflow {rows_used}")
    streams = []
    nslots = np.zeros(RANGES, np.int64)

    order = np.argsort(rloc_c, kind="stable")
    src_s = src_c[order]
    slot_s = slot_c[order]
    bounds = np.searchsorted(rloc_c[order], np.arange(RANGES + 1))
    for r in range(RANGES):
        lo, hi = bounds[r], bounds[r + 1]
        s = src_s[lo:hi]
        sl = slot_s[lo:hi]
        half_e = assignment[s]
        rows_e = rowof[s].astype(np.int64)
        mA = half_e == 0
        mB = ~mA
        rankA, urA, cntA = _ranks(rows_e[mA])
        rankB, urB, cntB = _ranks(rows_e[mB])
        ur = np.union1d(urA, urB)
        cA = np.zeros(len(ur), np.int64)
        cA[np.searchsorted(ur, urA)] = cntA
        cB = np.zeros(len(ur), np.int64)
        cB[np.searchsorted(ur, urB)] = cntB
        w = np.maximum(cA, cB)
        base = np.r_[0, np.cumsum(w)[:-1]]
        ns = int(w.sum())
        idx_r = np.repeat(ur, w).astype(np.int16)
        tgtA_r = np.full(ns, 255, np.int16)
        tgtA_r[base[np.searchsorted(ur, rows_e[mA])] + rankA] = sl[mA]
        tgtB_r = np.full(ns, 255, np.int16)
        tgtB_r[base[np.searchsorted(ur, rows_e[mB])] + rankB] = sl[mB]
        streams.append((idx_r, tgtA_r, tgtB_r))
        nslots[r] = ns
    return streams, nslots, assignment, rowof


def preprocess(x, edge_index):
    src = np.asarray(edge_index[0], dtype=np.int64)
    dst = np.asarray(edge_index[1], dtype=np.int64)
    deg = np.bincount(dst, minlength=N)
    recip = (1.0 / np.maximum(deg, 1)).astype(np.float32)

    bin_of_node, slot_of_node, loads = _bin_nodes(deg)
    ecore = bin_of_node[dst] // RANGES
    erloc = bin_of_node[dst] % RANGES
    eslot = slot_of_node[dst]

    xv = np.asarray(x, dtype=np.float32)
    per_core = []
    nslots_all = np.zeros((CORES, RANGES), np.int64)
    for c in range(CORES):
        m = ecore == c
        streams, nslots, assignment, rowof = _core_streams(
            src[m], erloc[m], eslot[m])
        per_core.append((streams, nslots, assignment, rowof))
        nslots_all[c] = nslots

    # per-core bin relabel (desc slot count) + shared block-count profile
    perms = [np.argsort(-nslots_all[c], kind="stable") for c in range(CORES)]
    sorted_ns = np.stack([nslots_all[c][perms[c]] for c in range(CORES)])
    profile = sorted_ns.max(axis=0)
    KP = np.maximum(np.ceil(profile / 128).astype(int), 1)
    if profile.max() > 2048:
        raise OverflowError(f"range overflow {profile.max()}")
    TOTBLK = int(KP.sum())
    groups = _make_groups(list(KP))

    cores = []
    for c in range(CORES):
        streams, nslots, assignment, rowof = per_core[c]
        perm = perms[c]
        idx_full = np.zeros((TOTBLK * 128,), np.int16)
        tgtA_full = np.full((TOTBLK * 128,), 255, np.int16)
        tgtB_full = np.full((TOTBLK * 128,), 255, np.int16)
        off = 0
        for r in range(RANGES):
            idx_r, tgtA_r, tgtB_r = streams[perm[r]]
            ns = len(idx_r)
            idx_full[off:off + ns] = idx_r
            tgtA_full[off:off + ns] = tgtA_r
            tgtB_full[off:off + ns] = tgtB_r
            off += KP[r] * 128
        call_streams = [idx_full[boff * 128:(boff + nblk) * 128]
                        for (_, _, boff, nblk) in groups]
        wrap = _wrap_idxs(call_streams)
        tgtA = np.ascontiguousarray(
            tgtA_full.reshape(TOTBLK, 128).T.astype(np.float32)).astype(NP_BF16)
        tgtB = np.ascontiguousarray(
            tgtB_full.reshape(TOTBLK, 128).T.astype(np.float32)).astype(NP_BF16)

        nodesA = np.where(assignment == 0)[0]
        nodesB = np.where(assignment == 1)[0]
        rsrcA = np.full(PAIR_ROWS, -1, np.int64)
        rsrcA[rowof[nodesA]] = nodesA
        rsrcB = np.full(PAIR_ROWS, -1, np.int64)
        rsrcB[rowof[nodesB]] = nodesB
        # own nodes in relabeled pos order
        own = np.full(NPAD, -1, np.int64)
        mc = bin_of_node // RANGES == c
        nodes_c = np.where(mc)[0]
        rinv = np.empty(RANGES, np.int64)
        rinv[perm] = np.arange(RANGES)
        own[rinv[bin_of_node[nodes_c] % RANGES] * 128
            + slot_of_node[nodes_c]] = nodes_c
        cores.append(dict(wrap=wrap, tgtA=tgtA, tgtB=tgtB,
                          rsrcA=rsrcA, rsrcB=rsrcB, own=own))

    def table_from(feats_by_node):
        out = []
        for c in range(CORES):
            t = np.zeros((PAIR_ROWS, 2 * D), NP_BF16)
            for half, key in ((0, "rsrcA"), (1, "rsrcB")):
                rs = cores[c][key]
                used = rs >= 0
                t[used, half * D:(half + 1) * D] = \
                    feats_by_node[rs[used]].astype(NP_BF16)
            out.append(t)
        return out

    xT = []
    recipb = []
    for c in range(CORES):
        own = cores[c]["own"]
        used = own >= 0
        t = np.zeros((NPAD, D), np.float32)
        t[used] = xv[own[used]]
        xT.append(np.ascontiguousarray(t.T).astype(NP_BF16))
        rb = np.zeros((NPAD,), np.float32)
        rb[used] = recip[own[used]]
        recipb.append(np.broadcast_to(rb.astype(NP_BF16), (128, NPAD)).copy())

    return cores, table_from, xT, recipb, tuple(KP.tolist()), xv


def kernel(x, edge_index, W1_l, b1, W1_r, W2_l, b2, W2_r, _timing=None):
    cores, table_from, xT, recipb, KP, xv = preprocess(x, edge_index)

    if KP not in _prog_cache:
        _prog_cache[KP] = (build_program(1, KP), build_program(2, KP))
    nc1, nc2 = _prog_cache[KP]

    def wmat(w):
        return np.asarray(w, dtype=np.float32).astype(NP_BF16)

    def bcol(b):
        return np.asarray(b, dtype=np.float32).reshape(128, 1)

    iota = np.ascontiguousarray(
        np.broadcast_to(np.arange(128, dtype=np.float32), (128, 128))
    ).astype(NP_BF16)
    tables1 = table_from(xv)
    maps1 = []
    for c in range(CORES):
        cc = cores[c]
        maps1.append(dict(table=tables1[c], idxs=cc["wrap"],
                          tgtA=cc["tgtA"], tgtB=cc["tgtB"], iota=iota,
                          xT=xT[c], recipb=recipb[c], Wl=wmat(W1_l),
                          Wr=wmat(W1_r), bvec=bcol(b1)))
    r1 = bass_utils.run_bass_kernel_spmd(nc1, maps1, core_ids=list(range(CORES)))

    # h by global node id (houts are feature-major in relabeled pos order)
    h_node = np.zeros((N, D), np.float32)
    for c in range(CORES):
        own = cores[c]["own"]
        used = own >= 0
        h_node[own[used]] = r1.results[c]["tout"].T[used]
    tables2 = table_from(h_node)

    maps2 = []
    for c in range(CORES):
        cc = cores[c]
        hT_own = np.asarray(r1.results[c]["tout"], dtype=np.float32).astype(NP_BF16)
        maps2.append(dict(table=tables2[c], idxs=cc["wrap"],
                          tgtA=cc["tgtA"], tgtB=cc["tgtB"], iota=iota,
                          xT=hT_own, recipb=recipb[c], Wl=wmat(W2_l),
                          Wr=wmat(W2_r), bvec=bcol(b2)))
    r2 = bass_utils.run_bass_kernel_spmd(nc2, maps2, core_ids=list(range(CORES)))
    if _timing is not None:
        _timing["nc1"] = nc1
        _timing["nc2"] = nc2

    out = np.empty((N, D), np.float32)
    for c in range(CORES):
        own = cores[c]["own"]
        used = own >= 0
        out[own[used]] = r2.results[c]["tout"].T[used]
    return out


# revision 40
# speedup vs baseline: 1.5560x; 1.1566x over previous
"""Two-layer SAGEConv (mean aggregation) GNN on 8 trn2 NeuronCores.

Strategy (dst-sharded graph parallel, "paired gather"):
  - dst nodes are packed into 320 bins of 128 slots (40 bins per core) by a
    degree-balanced snake-LPT, so every bin holds <= 2048 edges. Each bin is
    one psum "range".
  - Each core gets its OWN permuted pair-table [20480, 256] bf16: row k holds
    features of TWO source nodes (A-half cols 0:128, B-half cols 128:256).
    One 512-byte gather descriptor therefore serves up to two edges: sources
    are globally paired on their first-two-bins key (two rounds), and each
    128-edge slot routes its A-column value and its B-column value to
    independent dst slots. This cuts gather descriptors ~33% below one-per-
    edge, and 512B descriptors avoid the <512B 2x DMA latency penalty.
  - Per-bin slot counts are uneven, so bins are relabeled per core in
    descending slot count and the program is compiled for the shared
    per-range block-count profile (max across cores).
  - Routing matrices are generated ON-CHIP from slot-target bytes via
    broadcast is_equal on DVE in a slot-major layout (keeps last-dim
    stride-1 so the DVE 2x perf mode applies); generation is issued two
    gather-groups ahead and each group's lin/act phase is deferred behind
    the next group's agg matmuls, so neither the in-order DVE nor PE queue
    head-of-line blocks during the post-gather drain.
  - Aggregation per range: psum[f, slot] += sum_j msg[:, j, 0:128].T @ RA_j
    + msg[:, j, 128:256].T @ RB_j; then mean scale (1/deg along free dim),
    outT = act(W_l.T @ meanT + W_r.T @ xT + b), streamed out per group as
    feature-major [128, 5120] bf16 (the host transposes / re-permutes the h
    table between the two layer launches).
"""
import numpy as np
import ml_dtypes
from contextlib import ExitStack

import concourse.bass as bass
import concourse.mybir as mybir
import concourse.tile as tile
from concourse import bacc
from concourse.library_config import mlp
from concourse import bass_utils

BF16 = mybir.dt.bfloat16
F32 = mybir.dt.float32
I16 = mybir.dt.int16
NP_BF16 = ml_dtypes.bfloat16

N = 40000
D = 128
CORES = 8
RANGES = 40            # bins (dst ranges of 128 slots) per core
NPAD = RANGES * 128    # 5120 dst positions per core
NBINS = CORES * RANGES
PAIR_ROWS = 20480      # pair-table rows (int16-indexable)
GMAX = 32              # max message blocks per gather group (SBUF budget)

_prog_cache = {}


def _make_groups(KP):
    """Split program ranges into gather groups of <= GMAX blocks.
    Returns list of (range_lo, range_hi, block_offset, nblk). The final
    group is kept small (<= 2 ranges) to shorten the post-gather drain."""
    groups = []
    lo = 0
    off = 0
    cur = 0
    for r in range(RANGES - 2):
        if cur + KP[r] > GMAX and cur > 0:
            groups.append((lo, r, off, cur))
            off += cur
            lo = r
            cur = 0
        cur += KP[r]
    groups.append((lo, RANGES - 2, off, cur))
    off += cur
    tail = KP[RANGES - 2] + KP[RANGES - 1]
    groups.append((RANGES - 2, RANGES, off, tail))
    return groups


def build_program(layer, KP):
    """One SPMD program for one SAGEConv layer. KP[r] = pair-blocks of range r."""
    KP = list(KP)
    TOTBLK = sum(KP)
    IDX_COLS = TOTBLK * 8          # idx wrap cols (num_idxs/16 per call, concat)
    groups = _make_groups(KP)
    starts = np.concatenate([[0], np.cumsum(KP)]).astype(int)

    nc = bacc.Bacc("TRN2", target_bir_lowering=False, debug=False)
    table = nc.dram_tensor("table", [PAIR_ROWS, 2 * D], BF16, kind="ExternalInput")
    idx_d = nc.dram_tensor("idxs", [128, IDX_COLS], I16, kind="ExternalInput")
    tgtA_d = nc.dram_tensor("tgtA", [128, TOTBLK], BF16, kind="ExternalInput")
    tgtB_d = nc.dram_tensor("tgtB", [128, TOTBLK], BF16, kind="ExternalInput")
    iota_d = nc.dram_tensor("iota", [128, 128], BF16, kind="ExternalInput")
    xT_d = nc.dram_tensor("xT", [128, NPAD], BF16, kind="ExternalInput")
    recip_d = nc.dram_tensor("recipb", [1, NPAD], BF16, kind="ExternalInput")
    Wl_d = nc.dram_tensor("Wl", [128, 128], BF16, kind="ExternalInput")
    Wr_d = nc.dram_tensor("Wr", [128, 128], BF16, kind="ExternalInput")
    b_d = nc.dram_tensor("bvec", [128, 1], F32, kind="ExternalInput")
    # feature-major [f, pos]: host transposes (it re-permutes tables anyway)
    tout = nc.dram_tensor("tout", [128, NPAD], BF16, kind="ExternalOutput")

    with tile.TileContext(nc) as tc, ExitStack() as ctx:
        const = ctx.enter_context(tc.tile_pool(name="const", bufs=1))
        pmsg = ctx.enter_context(tc.tile_pool(name="msg", bufs=6))
        pohA = ctx.enter_context(tc.tile_pool(name="ohA", bufs=2))
        pohB = ctx.enter_context(tc.tile_pool(name="ohB", bufs=2))
        psagg = ctx.enter_context(tc.tile_pool(name="psagg", bufs=5, space="PSUM"))
        pslin = ctx.enter_context(tc.tile_pool(name="pslin", bufs=2, space="PSUM"))
        pmean = ctx.enter_context(tc.tile_pool(name="mean", bufs=10))
        psbc = ctx.enter_context(tc.tile_pool(name="psbc", bufs=1, space="PSUM"))

        nc.gpsimd.load_library(mlp)

        idxs = const.tile([128, IDX_COLS], I16)
        c0 = groups[0][3] * 8
        nc.sync.dma_start(idxs[:, :c0], idx_d[:, :c0])
        nc.sync.dma_start(idxs[:, c0:], idx_d[:, c0:])
        tgtA = const.tile([128, TOTBLK], BF16)
        nc.sync.dma_start(tgtA[:], tgtA_d[:])
        tgtB = const.tile([128, TOTBLK], BF16)
        nc.sync.dma_start(tgtB[:], tgtB_d[:])
        iota = const.tile([128, 128], BF16)
        nc.sync.dma_start(iota[:], iota_d[:])
        # iotaE[p, f, b] = f — stride-1 last dims for the DVE 2x perf mode
        iotaE = const.tile([128, 128, GMAX], BF16)
        nc.vector.tensor_copy(iotaE[:],
                              iota[:, :, None].to_broadcast([128, 128, GMAX]))
        xT = const.tile([128, NPAD], BF16)
        nc.sync.dma_start(xT[:], xT_d[:])
        # recip is a partition-broadcast of a 10KB row: fetch one row and
        # replicate it across partitions with K=1 matmuls against a ones
        # stationary (PE + Act are idle; DMA is the bottleneck).
        recipRow = const.tile([1, NPAD], BF16)
        nc.sync.dma_start(recipRow[:], recip_d[:])
        ones1 = const.tile([1, 128], BF16)
        nc.vector.memset(ones1[:], 1.0)
        zv = const.tile([128, 1], F32)
        nc.vector.memset(zv[:], 0.0)
        recip = const.tile([128, NPAD], BF16)
        for t in range(NPAD // 512):
            psb = psbc.tile([128, 512], F32)
            nc.tensor.matmul(psb[:], ones1[:],
                             recipRow[:, t * 512:(t + 1) * 512],
                             start=True, stop=True)
            nc.scalar.activation(recip[:, t * 512:(t + 1) * 512], psb[:],
                                 mybir.ActivationFunctionType.Identity,
                                 bias=zv[:])
        Wl = const.tile([128, 128], BF16)
        nc.sync.dma_start(Wl[:], Wl_d[:])
        Wr = const.tile([128, 128], BF16)
        nc.sync.dma_start(Wr[:], Wr_d[:])
        bv = const.tile([128, 1], F32)
        nc.sync.dma_start(bv[:], b_d[:])
        ostage = const.tile([128, NPAD], BF16)

        def gen_oh(pool_or_const, tgt, boff, nblk):
            oh = pool_or_const.tile([128, 128, nblk], BF16)
            nc.vector.tensor_tensor(
                out=oh[:],
                in0=tgt[:, None, boff:boff + nblk]
                .to_broadcast([128, 128, nblk]),
                in1=iotaE[:, :, :nblk],
                op=mybir.AluOpType.is_equal)
            return oh

        def emit_lin(rlo, rhi, means):
            for i, r in enumerate(range(rlo, rhi)):
                ps2 = pslin.tile([128, 128], F32)
                nc.tensor.matmul(ps2[:], Wl[:], means[i][:],
                                 start=True, stop=False)
                nc.tensor.matmul(ps2[:], Wr[:], xT[:, r * 128:(r + 1) * 128],
                                 start=False, stop=True)
                nc.scalar.activation(
                    ostage[:, r * 128:(r + 1) * 128], ps2[:],
                    mybir.ActivationFunctionType.Relu if layer == 1
                    else mybir.ActivationFunctionType.Identity,
                    bias=bv[:])
            # stream this group's outputs so the final write doesn't
            # serialize after the last gather
            nc.sync.dma_start(tout[:, rlo * 128:rhi * 128],
                              ostage[:, rlo * 128:rhi * 128])

        lin_work = None
        # DVE executes in issue order: generate routing matrices two groups
        # AHEAD of the compute that consumes the previous group's psums, so
        # gen is never head-of-line blocked behind the mean ops and the last
        # group's matmuls can start the moment its gather lands.
        from collections import deque
        pending = deque()
        for gi in range(min(2, len(groups))):
            g = groups[gi]
            pending.append((gen_oh(pohA, tgtA, g[2], g[3]),
                            gen_oh(pohB, tgtB, g[2], g[3])))
        for gi, (rlo, rhi, boff, nblk) in enumerate(groups):
            GN = nblk * 128
            msg = pmsg.tile([128, nblk, 2 * D], BF16)
            nc.gpsimd.dma_gather(msg[:], table[:, :],
                                 idxs[:, boff * 8:(boff + nblk) * 8],
                                 GN, GN, 2 * D, single_packet=False)
            ohA, ohB = pending.popleft()
            if gi + 2 < len(groups):
                nb = groups[gi + 2]
                pending.append((gen_oh(pohA, tgtA, nb[2], nb[3]),
                                gen_oh(pohB, tgtB, nb[2], nb[3])))

            # phase-batched + software-pipelined: issue this group's agg
            # matmuls and means now, but defer its lin/act/write phase until
            # after the NEXT group's agg matmuls are in the (in-order) PE
            # queue — so agg work never stalls behind lins waiting on DVE.
            pss = []
            for r in range(rlo, rhi):
                k = KP[r]
                ps = psagg.tile([128, 128], F32)
                for j in range(k):
                    bb = starts[r] - boff + j
                    nc.tensor.matmul(ps[:], msg[:, bb, 0:D],
                                     ohA[:, :, bb],
                                     start=(j == 0), stop=False)
                    nc.tensor.matmul(ps[:], msg[:, bb, D:2 * D],
                                     ohB[:, :, bb],
                                     start=False, stop=(j == k - 1))
                pss.append(ps)
            means = []
            for i, r in enumerate(range(rlo, rhi)):
                mean = pmean.tile([128, 128], BF16)
                nc.vector.tensor_mul(mean[:], pss[i][:],
                                     recip[:, r * 128:(r + 1) * 128])
                means.append(mean)
            if lin_work is not None:
                emit_lin(*lin_work)
            lin_work = (rlo, rhi, means)
        emit_lin(*lin_work)
    nc.compile()
    return nc


def _wrap_idxs(streams):
    """list of per-call idx streams (len % 2048 == 0) -> [128, sum/16] int16
    sbuf wrap layout (16-partition wrap per call, replicated to 128)."""
    cols = []
    for s in streams:
        a = s.reshape(-1, 16).T  # [16, GN/16]
        cols.append(a)
    a = np.concatenate(cols, axis=1)
    return np.tile(a, (8, 1)).astype(np.int16)


def _bin_nodes(deg):
    """Degree-balanced assignment of nodes to 320 bins of 128 slots."""
    order = np.argsort(-deg, kind="stable")
    loads = np.zeros(NBINS, np.int64)
    bin_of_node = np.empty(N, np.int64)
    slot_of_node = np.empty(N, np.int64)
    nrounds = (N + NBINS - 1) // NBINS
    for rnd in range(nrounds):
        chunk = order[rnd * NBINS:(rnd + 1) * NBINS]
        border = np.argsort(loads, kind="stable")[:len(chunk)]
        bin_of_node[chunk] = border
        slot_of_node[chunk] = rnd
        loads[border] += deg[chunk]
    return bin_of_node, slot_of_node, loads


def _ranks(rows):
    """Per-element rank within equal-value group of sorted-by-value `rows`,
    plus unique values and counts. rows need not be sorted."""
    o = np.argsort(rows, kind="stable")
    sr = rows[o]
    if len(sr) == 0:
        return np.empty(0, np.int64), np.empty(0, np.int64), np.empty(0, np.int64)
    newgrp = np.r_[True, sr[1:] != sr[:-1]]
    starts = np.flatnonzero(newgrp)
    grp = np.cumsum(newgrp) - 1
    pos = np.arange(len(sr)) - starts[grp]
    rank = np.empty(len(rows), np.int64)
    rank[o] = pos
    ur = sr[starts]
    cnt = np.diff(np.r_[starts, len(sr)])
    return rank, ur, cnt


def _greedy_pair(keys):
    """Pair elements (indices) having equal keys: returns (a_idx, b_idx,
    leftover_idx). Elements are paired consecutively within equal-key runs."""
    o = np.argsort(keys, kind="stable")
    ks = keys[o]
    if len(ks) == 0:
        z = np.empty(0, np.int64)
        return z, z, z
    newg = np.r_[True, ks[1:] != ks[:-1]]
    starts = np.flatnonzero(newg)
    gid = np.cumsum(newg) - 1
    pos = np.arange(len(ks)) - starts[gid]
    sizes = np.diff(np.r_[starts, len(ks)])
    odd_last = (pos == sizes[gid] - 1) & (sizes[gid] % 2 == 1)
    paired = ~odd_last
    po = o[paired]
    return po[0::2], po[1::2], o[odd_last]


def _pair_sources(src_c, rloc_c):
    """Global pairing of this core's sources by their first-two-bins key:
    a pair sharing two bins saves a gather slot in both. Returns
    (assignment [N] in {-1,0,1}, rowof [N], rows_used)."""
    key = src_c * 64 + rloc_c
    ub = np.unique(key)
    usrc, ubin = ub >> 6, ub & 63
    first = np.r_[True, usrc[1:] != usrc[:-1]]
    idx_first = np.flatnonzero(first)
    srcs_u = usrc[idx_first]
    b1 = ubin[idx_first]
    nxt_is_same = np.r_[idx_first[1:] - idx_first[:-1] > 1,
                        len(ub) - idx_first[-1] > 1]
    b2 = np.where(nxt_is_same, ubin[np.minimum(idx_first + 1, len(ub) - 1)], 64)
    # round 1: match on (first bin, second bin); round 2: leftovers on b1
    a1, bb1, left = _greedy_pair(b1 * 65 + b2)
    a2, bb2, left2 = _greedy_pair(b1[left])
    a_i = np.concatenate([a1, left[a2]])
    b_i = np.concatenate([bb1, left[bb2]])
    single = left[left2]

    assignment = np.full(N, -1, np.int8)
    rowof = np.zeros(N, np.int32)
    npairs = len(a_i)
    assignment[srcs_u[a_i]] = 0
    rowof[srcs_u[a_i]] = np.arange(npairs)
    assignment[srcs_u[b_i]] = 1
    rowof[srcs_u[b_i]] = np.arange(len(b_i))
    assignment[srcs_u[single]] = 0
    rowof[srcs_u[single]] = npairs + np.arange(len(single))
    return assignment, rowof, npairs + len(single)


def _core_streams(src_c, rloc_c, slot_c):
    """Per-core pair assignment + per-bin slot streams.

    Returns (streams: list of (idx_r, tgtA_r, tgtB_r) per physical bin,
    nslots [RANGES], assignment [N] in {-1,0,1}, rowof [N])."""
    assignment, rowof, rows_used = _pair_sources(src_c, rloc_c)
    if rows_used > PAIR_ROWS:
        raise OverflowError(f"pair rows overflow {rows_used}")
    streams = []
    nslots = np.zeros(RANGES, np.int64)

    order = np.argsort(rloc_c, kind="stable")
    src_s = src_c[order]
    slot_s = slot_c[order]
    bounds = np.searchsorted(rloc_c[order], np.arange(RANGES + 1))
    for r in range(RANGES):
        lo, hi = bounds[r], bounds[r + 1]
        s = src_s[lo:hi]
        sl = slot_s[lo:hi]
        half_e = assignment[s]
        rows_e = rowof[s].astype(np.int64)
        mA = half_e == 0
        mB = ~mA
        rankA, urA, cntA = _ranks(rows_e[mA])
        rankB, urB, cntB = _ranks(rows_e[mB])
        ur = np.union1d(urA, urB)
        cA = np.zeros(len(ur), np.int64)
        cA[np.searchsorted(ur, urA)] = cntA
        cB = np.zeros(len(ur), np.int64)
        cB[np.searchsorted(ur, urB)] = cntB
        w = np.maximum(cA, cB)
        base = np.r_[0, np.cumsum(w)[:-1]]
        ns = int(w.sum())
        idx_r = np.repeat(ur, w).astype(np.int16)
        tgtA_r = np.full(ns, 255, np.int16)
        tgtA_r[base[np.searchsorted(ur, rows_e[mA])] + rankA] = sl[mA]
        tgtB_r = np.full(ns, 255, np.int16)
        tgtB_r[base[np.searchsorted(ur, rows_e[mB])] + rankB] = sl[mB]
        streams.append((idx_r, tgtA_r, tgtB_r))
        nslots[r] = ns
    return streams, nslots, assignment, rowof


def preprocess(x, edge_index):
    src = np.asarray(edge_index[0], dtype=np.int64)
    dst = np.asarray(edge_index[1], dtype=np.int64)
    deg = np.bincount(dst, minlength=N)
    recip = (1.0 / np.maximum(deg, 1)).astype(np.float32)

    bin_of_node, slot_of_node, loads = _bin_nodes(deg)
    ecore = bin_of_node[dst] // RANGES
    erloc = bin_of_node[dst] % RANGES
    eslot = slot_of_node[dst]

    xv = np.asarray(x, dtype=np.float32)
    per_core = []
    nslots_all = np.zeros((CORES, RANGES), np.int64)
    for c in range(CORES):
        m = ecore == c
        streams, nslots, assignment, rowof = _core_streams(
            src[m], erloc[m], eslot[m])
        per_core.append((streams, nslots, assignment, rowof))
        nslots_all[c] = nslots

    # per-core bin relabel (desc slot count) + shared block-count profile
    perms = [np.argsort(-nslots_all[c], kind="stable") for c in range(CORES)]
    sorted_ns = np.stack([nslots_all[c][perms[c]] for c in range(CORES)])
    profile = sorted_ns.max(axis=0)
    KP = np.maximum(np.ceil(profile / 128).astype(int), 1)
    if profile.max() > 2048:
        raise OverflowError(f"range overflow {profile.max()}")
    TOTBLK = int(KP.sum())
    groups = _make_groups(list(KP))

    cores = []
    for c in range(CORES):
        streams, nslots, assignment, rowof = per_core[c]
        perm = perms[c]
        idx_full = np.zeros((TOTBLK * 128,), np.int16)
        tgtA_full = np.full((TOTBLK * 128,), 255, np.int16)
        tgtB_full = np.full((TOTBLK * 128,), 255, np.int16)
        off = 0
        for r in range(RANGES):
            idx_r, tgtA_r, tgtB_r = streams[perm[r]]
            ns = len(idx_r)
            idx_full[off:off + ns] = idx_r
            tgtA_full[off:off + ns] = tgtA_r
            tgtB_full[off:off + ns] = tgtB_r
            off += KP[r] * 128
        call_streams = [idx_full[boff * 128:(boff + nblk) * 128]
                        for (_, _, boff, nblk) in groups]
        wrap = _wrap_idxs(call_streams)
        tgtA = np.ascontiguousarray(
            tgtA_full.reshape(TOTBLK, 128).T.astype(np.float32)).astype(NP_BF16)
        tgtB = np.ascontiguousarray(
            tgtB_full.reshape(TOTBLK, 128).T.astype(np.float32)).astype(NP_BF16)

        nodesA = np.where(assignment == 0)[0]
        nodesB = np.where(assignment == 1)[0]
        rsrcA = np.full(PAIR_ROWS, -1, np.int64)
        rsrcA[rowof[nodesA]] = nodesA
        rsrcB = np.full(PAIR_ROWS, -1, np.int64)
        rsrcB[rowof[nodesB]] = nodesB
        # own nodes in relabeled pos order
        own = np.full(NPAD, -1, np.int64)
        mc = bin_of_node // RANGES == c
        nodes_c = np.where(mc)[0]
        rinv = np.empty(RANGES, np.int64)
        rinv[perm] = np.arange(RANGES)
        own[rinv[bin_of_node[nodes_c] % RANGES] * 128
            + slot_of_node[nodes_c]] = nodes_c
        cores.append(dict(wrap=wrap, tgtA=tgtA, tgtB=tgtB,
                          rsrcA=rsrcA, rsrcB=rsrcB, own=own))

    def table_from(feats_by_node):
        out = []
        for c in range(CORES):
            t = np.zeros((PAIR_ROWS, 2 * D), NP_BF16)
            for half, key in ((0, "rsrcA"), (1, "rsrcB")):
                rs = cores[c][key]
                used = rs >= 0
                t[used, half * D:(half + 1) * D] = \
                    feats_by_node[rs[used]].astype(NP_BF16)
            out.append(t)
        return out

    xT = []
    recipb = []
    for c in range(CORES):
        own = cores[c]["own"]
        used = own >= 0
        t = np.zeros((NPAD, D), np.float32)
        t[used] = xv[own[used]]
        xT.append(np.ascontiguousarray(t.T).astype(NP_BF16))
        rb = np.zeros((NPAD,), np.float32)
        rb[used] = recip[own[used]]
        recipb.append(rb.astype(NP_BF16).reshape(1, NPAD))

    return cores, table_from, xT, recipb, tuple(KP.tolist()), xv


def kernel(x, edge_index, W1_l, b1, W1_r, W2_l, b2, W2_r, _timing=None):
    cores, table_from, xT, recipb, KP, xv = preprocess(x, edge_index)

    if KP not in _prog_cache:
        _prog_cache[KP] = (build_program(1, KP), build_program(2, KP))
    nc1, nc2 = _prog_cache[KP]

    def wmat(w):
        return np.asarray(w, dtype=np.float32).astype(NP_BF16)

    def bcol(b):
        return np.asarray(b, dtype=np.float32).reshape(128, 1)

    iota = np.ascontiguousarray(
        np.broadcast_to(np.arange(128, dtype=np.float32), (128, 128))
    ).astype(NP_BF16)
    tables1 = table_from(xv)
    maps1 = []
    for c in range(CORES):
        cc = cores[c]
        maps1.append(dict(table=tables1[c], idxs=cc["wrap"],
                          tgtA=cc["tgtA"], tgtB=cc["tgtB"], iota=iota,
                          xT=xT[c], recipb=recipb[c], Wl=wmat(W1_l),
                          Wr=wmat(W1_r), bvec=bcol(b1)))
    r1 = bass_utils.run_bass_kernel_spmd(nc1, maps1, core_ids=list(range(CORES)))

    # h by global node id (houts are feature-major in relabeled pos order)
    h_node = np.zeros((N, D), np.float32)
    for c in range(CORES):
        own = cores[c]["own"]
        used = own >= 0
        h_node[own[used]] = r1.results[c]["tout"].T[used]
    tables2 = table_from(h_node)

    maps2 = []
    for c in range(CORES):
        cc = cores[c]
        hT_own = np.asarray(r1.results[c]["tout"], dtype=np.float32).astype(NP_BF16)
        maps2.append(dict(table=tables2[c], idxs=cc["wrap"],
                          tgtA=cc["tgtA"], tgtB=cc["tgtB"], iota=iota,
                          xT=hT_own, recipb=recipb[c], Wl=wmat(W2_l),
                          Wr=wmat(W2_r), bvec=bcol(b2)))
    r2 = bass_utils.run_bass_kernel_spmd(nc2, maps2, core_ids=list(range(CORES)))
    if _timing is not None:
        _timing["nc1"] = nc1
        _timing["nc2"] = nc2

    out = np.empty((N, D), np.float32)
    for c in range(CORES):
        own = cores[c]["own"]
        used = own >= 0
        out[own[used]] = r2.results[c]["tout"].T[used]
    return out
